# revision 16
# baseline (speedup 1.0000x reference)
"""Multi-head causal self-attention on 8 Trainium2 NeuronCores.

Problem: x[4,2048,1024] @ w_qkv[1024,3072] -> 16-head causal attention
         -> @ w_out[1024,1024] + b_out.

Sharding (hardcoded): 8 cores = 4 batches x 2 head-groups of 8 heads.
Core c handles batch b = c//2 and heads hg*8..hg*8+8, hg = c%2.
Each core computes a partial output [2048,1024] (its 8 heads pushed
through its w_out row-slice); host sums the two head-group partials per
batch and adds b_out.

Everything computes in fp16 (10 mantissa bits; fp32 PSUM accumulation),
which runs matmuls at full 1 cycle/row PE rate and lands ~7e-4 relative
error vs the fp32 reference.

Device algorithm per core (all "transposed orientation" so the only
transpose needed -- x^T -- is done for free on the host):
  qT/kT [512, 2048] and v (natural [2048, 512]) via fp16 matmuls.
  Per head pair (2 heads = 128 partitions), per 512-wide query chunk:
    scores^T[j,i] for both heads into one 2-bank PSUM tile via
    row-tiled (K=64) matmul pairs; ONE exp per key-tile on ScalarE
    (p^T fp16); causal masking via a precomputed 0/1 mask multiply on
    the diagonal band plus variable-width (narrowed) tiles;
    out^T[d,i] += col-tiled matmuls (PSUM accum over j),
    denom[i]   += ones-vector matmuls (M=1) into shared denom banks
    (4 col-strip rows per bank, zero-established by a dummy matmul).
  att^T (unnormalized) is copied to SBUF immediately (frees PSUM);
  1/denom via one batched DVE reciprocal per bank, broadcast over
  partitions via a DRAM bounce, then in-place multiply into att^T.
  partial = att^T.T @ w_out_slice -> DMA to DRAM.

Emission is software-pipelined per 512-token stage s: QKV(s),
out-projection(s-1), attention(s), so the Tile scheduler overlaps
PE-heavy projection work with ScalarE-heavy softmax work and hides the
softmax-denominator normalization latency.
"""

import os
import sys

import numpy as np

if "/opt/trn_rl_repo" not in sys.path:
    sys.path.insert(0, "/opt/trn_rl_repo")

B, T, C = 4, 2048, 1024
H, D = 16, 64
NCORES = 8
HPC = 8  # heads per core
PAIRS = 4  # head pairs per core
CCH = 8  # contraction chunks over C (1024/128)
ICH = 4  # i (query) chunks of 512
NJT = 16  # j (key) tiles of 128

_CACHE = {}


def _build_program():
    import concourse.mybir as mybir
    import concourse.tile as tile
    from concourse import bacc

    f32 = mybir.dt.float32
    f32r = mybir.dt.float32r
    bf16 = mybir.dt.bfloat16
    f16 = mybir.dt.float16
    EXP = mybir.ActivationFunctionType.Exp

    nc = bacc.Bacc(
        "TRN2", target_bir_lowering=False, debug=False, num_devices=NCORES
    )
    xt = nc.dram_tensor("xt", [C, T], f16, kind="ExternalInput").ap()
    wq = nc.dram_tensor("wq", [C, 512], f16, kind="ExternalInput").ap()
    wk = nc.dram_tensor("wk", [C, 512], f16, kind="ExternalInput").ap()
    wv = nc.dram_tensor("wv", [C, 512], f16, kind="ExternalInput").ap()
    wo = nc.dram_tensor("wo", [512, C], f16, kind="ExternalInput").ap()
    msk = nc.dram_tensor("msk", [128, 896], f16, kind="ExternalInput").ap()
    out = nc.dram_tensor("out", [T, C], f32, kind="ExternalOutput").ap()

    with tile.TileContext(nc) as tc:
        with (
            tc.tile_pool(name="wpool", bufs=16) as wpool,
            tc.tile_pool(name="wvpool", bufs=8) as wvpool,
            tc.tile_pool(name="wopool", bufs=4) as wopool,
            tc.tile_pool(name="xpool", bufs=8) as xpool,
            tc.tile_pool(name="qkpool", bufs=8) as qkpool,
            tc.tile_pool(name="vpool", bufs=16) as vpool,
            tc.tile_pool(name="apool", bufs=4) as apool,
            tc.tile_pool(name="ppool", bufs=12) as ppool,
            tc.tile_pool(name="cpool", bufs=1) as cpool,
            tc.tile_pool(name="rpool", bufs=4) as rpool,
            tc.tile_pool(name="qpool", bufs=4) as qpool,
            tc.tile_pool(name="opool", bufs=4) as opool,
            tc.tile_pool(name="dpool", bufs=4, space="DRAM") as dpool,
            tc.tile_pool(name="ps_a", bufs=2, space="PSUM") as ps_a,
            tc.tile_pool(name="ps_s", bufs=2, space="PSUM") as ps_s,
            tc.tile_pool(name="ps_o", bufs=2, space="PSUM") as ps_o,
        ):
            # ---- constants / weights resident in SBUF ----
            mask_sb = cpool.tile([128, 896], f16, name="mask_sb")
            nc.sync.dma_start(out=mask_sb, in_=msk)
            ones_sb = cpool.tile([128, 1], f16, name="ones_sb")
            nc.vector.memset(ones_sb, 1.0)

            w_sb = {}

            def load_w(wname, wap):
                for cc in range(CCH):
                    t = wpool.tile(
                        [128, 512], f16, name=f"{wname}_{cc}", tag="w"
                    )
                    nc.sync.dma_start(
                        out=t, in_=wap[cc * 128 : (cc + 1) * 128, :]
                    )
                    w_sb[wname, cc] = t

            # First compute needs wq + x^T chunk 0. Interleave those DMAs
            # at [128,512] granularity so the first matmul group can start
            # after ~2 transfers, then stream the rest in compute order.
            xt_sb = [
                xpool.tile([128, T], f16, name=f"xt_{cc}", tag="xt")
                for cc in range(CCH)
            ]

            def load_xt_chunk(s):
                tsl = slice(s * 512, (s + 1) * 512)
                for cc in range(CCH):
                    nc.sync.dma_start(
                        out=xt_sb[cc][:, tsl],
                        in_=xt[cc * 128 : (cc + 1) * 128, tsl],
                    )

            for cc in range(CCH):
                t = wpool.tile([128, 512], f16, name=f"wq_{cc}", tag="w")
                nc.sync.dma_start(out=t, in_=wq[cc * 128 : (cc + 1) * 128, :])
                w_sb["wq", cc] = t
                nc.sync.dma_start(
                    out=xt_sb[cc][:, 0:512],
                    in_=xt[cc * 128 : (cc + 1) * 128, 0:512],
                )
            load_w("wk", wk)
            for cc in range(CCH):
                t = wvpool.tile([128, 512], f16, name=f"wv_{cc}", tag="wv")
                nc.sync.dma_start(out=t, in_=wv[cc * 128 : (cc + 1) * 128, :])
                w_sb["wv", cc] = t
            load_xt_chunk(1)
            wo_sb = []
            for fc in range(4):
                t = wopool.tile([128, C], f16, name=f"wo_{fc}", tag="wo")
                nc.sync.dma_start(out=t, in_=wo[fc * 128 : (fc + 1) * 128, :])
                wo_sb.append(t)
            load_xt_chunk(2)
            load_xt_chunk(3)

            # ---- persistent activations ----
            qT = [
                qkpool.tile([128, T], f16, name=f"qT_{p}", tag="qk")
                for p in range(PAIRS)
            ]
            kT = [
                qkpool.tile([128, T], f16, name=f"kT_{p}", tag="qk")
                for p in range(PAIRS)
            ]
            v_sb = [
                vpool.tile([128, 512], f16, name=f"v_{j}", tag="v")
                for j in range(NJT)
            ]
            att = [
                apool.tile([128, T], f16, name=f"att_{p}", tag="att")
                for p in range(PAIRS)
            ]

            def phase_a(t4):
                """QKV projections for token chunk t4 (512 tokens)."""
                tsl4 = slice(t4 * 512, (t4 + 1) * 512)
                xts = [xt_sb[cc][:, tsl4] for cc in range(CCH)]
                for wname, dst in (("wq", qT), ("wk", kT)):
                    for n in range(PAIRS):
                        ps = ps_a.tile([128, 512], f32, name="ps_qk", tag="psA")
                        for cc in range(CCH):
                            nc.tensor.matmul(
                                ps,
                                lhsT=w_sb[wname, cc][:, n * 128 : (n + 1) * 128],
                                rhs=xts[cc][:],
                                start=(cc == 0),
                                stop=(cc == CCH - 1),
                            )
                        nc.vector.tensor_copy(
                            dst[n][:, t4 * 512 : (t4 + 1) * 512], ps
                        )
                for tt in range(4):
                    ps = ps_a.tile([128, 512], f32, name="ps_v", tag="psA")
                    for cc in range(CCH):
                        nc.tensor.matmul(
                            ps,
                            lhsT=xts[cc][:, tt * 128 : (tt + 1) * 128],
                            rhs=w_sb["wv", cc][:],
                            start=(cc == 0),
                            stop=(cc == CCH - 1),
                        )
                    nc.vector.tensor_copy(v_sb[t4 * 4 + tt], ps)

            def phase_b(ic):
                """Attention for query chunk ic (512 queries).

                Pairs run as two interleaved duos (0,1) then (2,3): the
                j-tile loops of the duo alternate at emission so one pair's
                PV work hides the other pair's exp latency and the PE never
                idles long enough to drop out of its fast p-state.
                """
                isl = slice(ic * 512, (ic + 1) * 512)
                njt = 4 * ic + 4

                def norm_pair(pr, den):
                    """1/denominators for pair pr -> rdb + in-place mul."""
                    rec = rpool.tile([128, 1024], f32, name="rec", tag="rec")
                    nc.vector.tensor_copy(rec[0:33, 0:512], den[0:33, 0:512])
                    nc.vector.reciprocal_approx_fast(
                        rec[0:33, 512:1024], rec[0:33, 0:512]
                    )
                    dsc = dpool.tile([2, 512], f32, name="dsc", tag="dsc")
                    nc.sync.dma_start(out=dsc, in_=rec[0:33:32, 512:1024])
                    rdb = rpool.tile([128, 512], f32, name="rdb", tag="rdb")
                    nc.sync.dma_start(
                        out=rdb[0:64, :],
                        in_=dsc[0:1, :].broadcast_to([64, 512]),
                    )
                    nc.sync.dma_start(
                        out=rdb[64:128, :],
                        in_=dsc[1:2, :].broadcast_to([64, 512]),
                    )
                    asl = att[pr][:, isl]
                    nc.vector.tensor_mul(asl, asl, rdb)

                def qk_exp(pr, jt, sb, pTb):
                    jsl = slice(jt * 128, (jt + 1) * 128)
                    dpos = jt - 4 * ic
                    # Causal: query columns below 128*dpos within this chunk
                    # see none of this key tile, so both QK matmuls narrow
                    # to the valid query range. Head 1's scores land at
                    # column 512 (adjacent to head 0's valid region) so one
                    # exp covers both halves with no dead zone.
                    ioff = 128 * dpos if dpos > 0 else 0
                    w = 512 - ioff
                    islw = slice(ic * 512 + ioff, (ic + 1) * 512)
                    nc.tensor.matmul(
                        sb[:, ioff:512],
                        lhsT=kT[pr][0:64, jsl],
                        rhs=qT[pr][0:64, islw],
                        start=True,
                        stop=True,
                        tile_position=(0, 0),
                    )
                    nc.tensor.matmul(
                        sb[:, 512 : 512 + w],
                        lhsT=kT[pr][64:128, jsl],
                        rhs=qT[pr][64:128, islw],
                        start=True,
                        stop=True,
                        tile_position=(64, 0),
                    )
                    nc.scalar.activation(
                        pTb[:, ioff : 512 + w],
                        sb[:, ioff : 512 + w],
                        EXP,
                        scale=0.125,
                    )
                    if dpos >= 0:
                        msl = mask_sb[:, 384 : 384 + w]
                        nc.vector.tensor_mul(
                            pTb[:, ioff:512], pTb[:, ioff:512], msl
                        )
                        nc.vector.tensor_mul(
                            pTb[:, 512 : 512 + w], pTb[:, 512 : 512 + w], msl
                        )
                    return ioff, w

                def pv_acc(pr, jt, pTb, ioff, w, ps_out, pacc0, pacc1):
                    first = jt == 0
                    last = jt == njt - 1
                    vt = v_sb[jt]
                    pT0 = pTb[:, ioff:512]
                    pT1 = pTb[:, 512 : 512 + w]
                    nc.tensor.matmul(
                        ps_out[0:64, ioff:512],
                        lhsT=vt[:, pr * 128 : pr * 128 + 64],
                        rhs=pT0,
                        start=first,
                        stop=False,
                        tile_position=(0, 0),
                        skip_group_check=True,
                    )
                    nc.tensor.matmul(
                        ps_out[64:128, ioff:512],
                        lhsT=vt[:, pr * 128 + 64 : pr * 128 + 128],
                        rhs=pT1,
                        start=first,
                        stop=last,
                        tile_position=(0, 64),
                        skip_group_check=True,
                    )
                    if first:
                        nc.vector.tensor_copy(pacc0, pTb[:, 0:512])
                        nc.vector.tensor_copy(pacc1, pTb[:, 512:1024])
                    else:
                        nc.vector.tensor_add(
                            pacc0[:, ioff:512], pacc0[:, ioff:512], pT0
                        )
                        nc.vector.tensor_add(
                            pacc1[:, ioff:512], pacc1[:, ioff:512], pT1
                        )

                for g in range(2):
                    duo = (2 * g, 2 * g + 1)
                    ps_outs = {}
                    paccs = {}
                    for pr in duo:
                        ps_outs[pr] = ps_o.tile(
                            [128, 512], f32, name="ps_out", tag="pso"
                        )
                        paccs[pr] = (
                            qpool.tile([128, 512], f16, name="pacc0", tag="pacc"),
                            qpool.tile([128, 512], f16, name="pacc1", tag="pacc"),
                        )
                    for jt in range(njt):
                        for pr in duo:
                            sb = ps_s.tile([128, 1024], f32, name="sb", tag="pss")
                            pTb = ppool.tile(
                                [128, 1024], f16, name="pTb", tag="pT"
                            )
                            ioff, w = qk_exp(pr, jt, sb, pTb)
                            pv_acc(pr, jt, pTb, ioff, w, ps_outs[pr], *paccs[pr])
                    for pr in duo:
                        # Partition-reduce the accumulated p-sums into rows
                        # 0/32 of a retired score-ring slot (frees a
                        # dedicated denominator bank).
                        den = ps_s.tile([128, 1024], f32, name="den", tag="pss")
                        nc.tensor.matmul(
                            den[0:1, 0:512],
                            lhsT=ones_sb,
                            rhs=paccs[pr][0],
                            start=True,
                            stop=True,
                            tile_position=(0, 0),
                            skip_group_check=True,
                        )
                        nc.tensor.matmul(
                            den[32:33, 0:512],
                            lhsT=ones_sb,
                            rhs=paccs[pr][1],
                            start=True,
                            stop=True,
                            tile_position=(0, 32),
                            skip_group_check=True,
                        )
                        # Unnormalized copy frees ps_out quickly;
                        # normalization happens in-place on att once the
                        # broadcast lands.
                        asl = att[pr][:, isl]
                        nc.vector.tensor_copy(asl, ps_outs[pr])
                        norm_pair(pr, den)

            def phase_c(s):
                """Output projection for token tiles 4s..4s+4."""
                for tt in range(4 * s, 4 * s + 4):
                    tsl = slice(tt * 128, (tt + 1) * 128)
                    for n in range(2):
                        ps = ps_a.tile([128, 512], f32, name="ps_c", tag="psA")
                        for fc in range(4):
                            nc.tensor.matmul(
                                ps,
                                lhsT=att[fc][:, tsl],
                                rhs=wo_sb[fc][:, n * 512 : (n + 1) * 512],
                                start=(fc == 0),
                                stop=(fc == 3),
                            )
                        ost = opool.tile([128, 512], f32, name="ost", tag="ost")
                        nc.vector.tensor_copy(ost, ps)
                        nc.sync.dma_start(
                            out=out[tsl, n * 512 : (n + 1) * 512], in_=ost
                        )

            # Emission order interleaves so every B phase has PE filler
            # work available: B(s) overlaps A(s+1) (emitted just before)
            # and C(s-1) (emitted just after B(s) starts). B(3), the
            # longest attention chunk, gets C(2)+C(3) as filler.
            phase_a(0)
            phase_b(0)
            phase_a(1)
            phase_b(1)
            phase_c(0)
            phase_a(2)
            phase_b(2)
            phase_c(1)
            phase_a(3)
            phase_b(3)
            phase_c(2)
            phase_c(3)

    nc.compile()
    return nc


def _get_program():
    if "nc" not in _CACHE:
        _CACHE["nc"] = _build_program()
    return _CACHE["nc"]


def _make_mask():
    # msk[jj, z] = 1 if z >= jj + 384 else 0; diagonal-position-p mask
    # tile is msk[:, 384-128p : 384-128p+512].
    jj = np.arange(128)[:, None]
    z = np.arange(896)[None, :]
    return (z >= jj + 384).astype(np.float16)


def _make_in_maps(x, w_qkv, w_out):
    mask = _make_mask()
    in_maps = []
    for core in range(NCORES):
        b, hg = core // 2, core % 2
        cs = slice(hg * 512, (hg + 1) * 512)
        f16 = np.float16
        in_maps.append(
            {
                "xt": np.ascontiguousarray(x[b].T).astype(f16),
                "wq": np.ascontiguousarray(
                    w_qkv[:, hg * 512 : hg * 512 + 512]
                ).astype(f16),
                "wk": np.ascontiguousarray(
                    w_qkv[:, 1024 + hg * 512 : 1024 + hg * 512 + 512]
                ).astype(f16),
                "wv": np.ascontiguousarray(
                    w_qkv[:, 2048 + hg * 512 : 2048 + hg * 512 + 512]
                ).astype(f16),
                "wo": np.ascontiguousarray(w_out[cs, :]).astype(f16),
                "msk": mask,
            }
        )
    return in_maps


def _run_device(in_maps, trace=False):
    from concourse.bass_utils import run_bass_kernel_spmd

    nc = _get_program()
    return run_bass_kernel_spmd(
        nc, in_maps, core_ids=list(range(NCORES)), trace=trace
    )


def kernel(x, w_qkv, w_out, b_out):
    x = np.asarray(x, dtype=np.float32)
    w_qkv = np.asarray(w_qkv, dtype=np.float32)
    w_out = np.asarray(w_out, dtype=np.float32)
    b_out = np.asarray(b_out, dtype=np.float32)

    res = _run_device(_make_in_maps(x, w_qkv, w_out)).results
    out = np.empty((B, T, C), dtype=np.float32)
    for b in range(B):
        out[b] = res[2 * b]["out"] + res[2 * b + 1]["out"] + b_out
    return out



# revision 20
# speedup vs baseline: 1.0095x; 1.0095x over previous
"""Multi-head causal self-attention on 8 Trainium2 NeuronCores.

Problem: x[4,2048,1024] @ w_qkv[1024,3072] -> 16-head causal attention
         -> @ w_out[1024,1024] + b_out.

Sharding (hardcoded): 8 cores = 4 batches x 2 head-groups of 8 heads.
Core c handles batch b = c//2 and heads hg*8..hg*8+8, hg = c%2.
Each core computes a partial output [2048,1024] (its 8 heads pushed
through its w_out row-slice); host sums the two head-group partials per
batch and adds b_out.

Everything computes in fp16 (10 mantissa bits; fp32 PSUM accumulation),
which runs matmuls at full 1 cycle/row PE rate and lands ~7e-4 relative
error vs the fp32 reference.

Device algorithm per core (all "transposed orientation" so the only
transpose needed -- x^T -- is done for free on the host):
  qT/kT [512, 2048] and v (natural [2048, 512]) via fp16 matmuls.
  Per head pair (2 heads = 128 partitions), per 512-wide query chunk:
    scores^T[j,i] for both heads into one 2-bank PSUM tile via
    row-tiled (K=64) matmul pairs; ONE exp per key-tile on ScalarE
    (p^T fp16); causal masking via a precomputed 0/1 mask multiply on
    the diagonal band plus variable-width (narrowed) tiles;
    out^T[d,i] += col-tiled matmuls (PSUM accum over j),
    denom[i]   += ones-vector matmuls (M=1) into shared denom banks
    (4 col-strip rows per bank, zero-established by a dummy matmul).
  att^T (unnormalized) is copied to SBUF immediately (frees PSUM);
  1/denom via one batched DVE reciprocal per bank, broadcast over
  partitions via a DRAM bounce, then in-place multiply into att^T.
  partial = att^T.T @ w_out_slice -> DMA to DRAM.

Emission is software-pipelined per 512-token stage s: QKV(s),
out-projection(s-1), attention(s), so the Tile scheduler overlaps
PE-heavy projection work with ScalarE-heavy softmax work and hides the
softmax-denominator normalization latency.
"""

import os
import sys

import numpy as np

if "/opt/trn_rl_repo" not in sys.path:
    sys.path.insert(0, "/opt/trn_rl_repo")

B, T, C = 4, 2048, 1024
H, D = 16, 64
NCORES = 8
HPC = 8  # heads per core
PAIRS = 4  # head pairs per core
CCH = 8  # contraction chunks over C (1024/128)
ICH = 4  # i (query) chunks of 512
NJT = 16  # j (key) tiles of 128

_CACHE = {}


def _build_program():
    import concourse.mybir as mybir
    import concourse.tile as tile
    from concourse import bacc

    f32 = mybir.dt.float32
    f32r = mybir.dt.float32r
    bf16 = mybir.dt.bfloat16
    f16 = mybir.dt.float16
    EXP = mybir.ActivationFunctionType.Exp

    nc = bacc.Bacc(
        "TRN2", target_bir_lowering=False, debug=False, num_devices=NCORES
    )
    xt = nc.dram_tensor("xt", [C, T], f16, kind="ExternalInput").ap()
    wq = nc.dram_tensor("wq", [C, 512], f16, kind="ExternalInput").ap()
    wk = nc.dram_tensor("wk", [C, 512], f16, kind="ExternalInput").ap()
    wv = nc.dram_tensor("wv", [C, 512], f16, kind="ExternalInput").ap()
    wo = nc.dram_tensor("wo", [512, C], f16, kind="ExternalInput").ap()
    msk = nc.dram_tensor("msk", [128, 896], f16, kind="ExternalInput").ap()
    out = nc.dram_tensor("out", [T, C], f16, kind="ExternalOutput").ap()

    with tile.TileContext(nc) as tc:
        with (
            tc.tile_pool(name="wpool", bufs=2) as wpool,
            tc.tile_pool(name="wvpool", bufs=1) as wvpool,
            tc.tile_pool(name="wopool", bufs=1) as wopool,
            tc.tile_pool(name="xpool", bufs=1) as xpool,
            tc.tile_pool(name="qkpool", bufs=8) as qkpool,
            tc.tile_pool(name="vpool", bufs=16) as vpool,
            tc.tile_pool(name="apool", bufs=4) as apool,
            tc.tile_pool(name="ppool", bufs=12) as ppool,
            tc.tile_pool(name="cpool", bufs=1) as cpool,
            tc.tile_pool(name="rpool", bufs=4) as rpool,
            tc.tile_pool(name="qpool", bufs=4) as qpool,
            tc.tile_pool(name="opool", bufs=4) as opool,
            tc.tile_pool(name="dpool", bufs=4, space="DRAM") as dpool,
            tc.tile_pool(name="ps_a", bufs=2, space="PSUM") as ps_a,
            tc.tile_pool(name="ps_s", bufs=2, space="PSUM") as ps_s,
            tc.tile_pool(name="ps_o", bufs=2, space="PSUM") as ps_o,
        ):
            # ---- constants / weights resident in SBUF ----
            mask_sb = cpool.tile([128, 896], f16, name="mask_sb")
            nc.sync.dma_start(out=mask_sb, in_=msk)
            ones_sb = cpool.tile([128, 1], f16, name="ones_sb")
            nc.vector.memset(ones_sb, 1.0)

            # Merged persistent input tiles, loaded with one big DMA per
            # tensor (or per 512-token chunk for x^T), split across the two
            # hardware DMA queues (Sync + Activation) so the first matmul
            # group is fed after ~2 transfers.
            xt_all = xpool.tile([128, CCH * T], f16, name="xt_all", tag="xt")
            wq_all = wpool.tile([128, CCH * 512], f16, name="wq_all", tag="w")
            wk_all = wpool.tile([128, CCH * 512], f16, name="wk_all", tag="w")
            wv_all = wvpool.tile([128, CCH * 512], f16, name="wv_all", tag="wv")
            wo_all = wopool.tile([128, 4 * C], f16, name="wo_all", tag="wo")

            xt_r = xt.rearrange("(c p) t -> p c t", p=128)
            xt_v = xt_all.rearrange("p (c t) -> p c t", c=CCH)

            def load_xt_chunk(s):
                ssl = slice(s * 512, (s + 1) * 512)
                nc.scalar.dma_start(out=xt_v[:, :, ssl], in_=xt_r[:, :, ssl])

            load_xt_chunk(0)
            nc.sync.dma_start(
                out=wq_all.rearrange("p (c j) -> p c j", c=CCH),
                in_=wq.rearrange("(c p) j -> p c j", p=128),
            )
            nc.sync.dma_start(
                out=wk_all.rearrange("p (c j) -> p c j", c=CCH),
                in_=wk.rearrange("(c p) j -> p c j", p=128),
            )
            load_xt_chunk(1)
            nc.sync.dma_start(
                out=wv_all.rearrange("p (c j) -> p c j", c=CCH),
                in_=wv.rearrange("(c p) j -> p c j", p=128),
            )
            nc.sync.dma_start(
                out=wo_all.rearrange("p (f j) -> p f j", f=4),
                in_=wo.rearrange("(f p) j -> p f j", p=128),
            )
            load_xt_chunk(2)
            load_xt_chunk(3)

            w_sb = {}
            for cc in range(CCH):
                w_sb["wq", cc] = wq_all[:, cc * 512 : (cc + 1) * 512]
                w_sb["wk", cc] = wk_all[:, cc * 512 : (cc + 1) * 512]
                w_sb["wv", cc] = wv_all[:, cc * 512 : (cc + 1) * 512]
            wo_sb = [wo_all[:, fc * C : (fc + 1) * C] for fc in range(4)]
            xt_sb = [xt_all[:, cc * T : (cc + 1) * T] for cc in range(CCH)]

            # ---- persistent activations ----
            qT = [
                qkpool.tile([128, T], f16, name=f"qT_{p}", tag="qk")
                for p in range(PAIRS)
            ]
            kT = [
                qkpool.tile([128, T], f16, name=f"kT_{p}", tag="qk")
                for p in range(PAIRS)
            ]
            v_sb = [
                vpool.tile([128, 512], f16, name=f"v_{j}", tag="v")
                for j in range(NJT)
            ]
            att = [
                apool.tile([128, T], f16, name=f"att_{p}", tag="att")
                for p in range(PAIRS)
            ]

            def phase_a(t4):
                """QKV projections for token chunk t4 (512 tokens)."""
                tsl4 = slice(t4 * 512, (t4 + 1) * 512)
                xts = [xt_sb[cc][:, tsl4] for cc in range(CCH)]
                for wname, dst in (("wq", qT), ("wk", kT)):
                    for n in range(PAIRS):
                        ps = ps_a.tile([128, 512], f32, name="ps_qk", tag="psA")
                        for cc in range(CCH):
                            nc.tensor.matmul(
                                ps,
                                lhsT=w_sb[wname, cc][:, n * 128 : (n + 1) * 128],
                                rhs=xts[cc][:],
                                start=(cc == 0),
                                stop=(cc == CCH - 1),
                            )
                        nc.vector.tensor_copy(
                            dst[n][:, t4 * 512 : (t4 + 1) * 512], ps
                        )
                for tt in range(4):
                    ps = ps_a.tile([128, 512], f32, name="ps_v", tag="psA")
                    for cc in range(CCH):
                        nc.tensor.matmul(
                            ps,
                            lhsT=xts[cc][:, tt * 128 : (tt + 1) * 128],
                            rhs=w_sb["wv", cc][:],
                            start=(cc == 0),
                            stop=(cc == CCH - 1),
                        )
                    nc.vector.tensor_copy(v_sb[t4 * 4 + tt], ps)

            def phase_b(ic):
                """Attention for query chunk ic (512 queries).

                Pairs run as two interleaved duos (0,1) then (2,3): the
                j-tile loops of the duo alternate at emission so one pair's
                PV work hides the other pair's exp latency and the PE never
                idles long enough to drop out of its fast p-state.
                """
                isl = slice(ic * 512, (ic + 1) * 512)
                njt = 4 * ic + 4

                def norm_pair(pr, den):
                    """1/denominators for pair pr -> rdb + in-place mul."""
                    rec = rpool.tile([128, 1024], f32, name="rec", tag="rec")
                    nc.vector.tensor_copy(rec[0:33, 0:512], den[0:33, 0:512])
                    nc.vector.reciprocal_approx_fast(
                        rec[0:33, 512:1024], rec[0:33, 0:512]
                    )
                    dsc = dpool.tile([2, 512], f32, name="dsc", tag="dsc")
                    nc.sync.dma_start(out=dsc, in_=rec[0:33:32, 512:1024])
                    rdb = rpool.tile([128, 512], f32, name="rdb", tag="rdb")
                    nc.sync.dma_start(
                        out=rdb[0:64, :],
                        in_=dsc[0:1, :].broadcast_to([64, 512]),
                    )
                    nc.sync.dma_start(
                        out=rdb[64:128, :],
                        in_=dsc[1:2, :].broadcast_to([64, 512]),
                    )
                    asl = att[pr][:, isl]
                    nc.vector.tensor_mul(asl, asl, rdb)

                def qk_exp(pr, jt, sb, pTb):
                    jsl = slice(jt * 128, (jt + 1) * 128)
                    dpos = jt - 4 * ic
                    # Causal: query columns below 128*dpos within this chunk
                    # see none of this key tile, so both QK matmuls narrow
                    # to the valid query range. Head 1's scores land at
                    # column 512 (adjacent to head 0's valid region) so one
                    # exp covers both halves with no dead zone.
                    ioff = 128 * dpos if dpos > 0 else 0
                    w = 512 - ioff
                    islw = slice(ic * 512 + ioff, (ic + 1) * 512)
                    nc.tensor.matmul(
                        sb[:, ioff:512],
                        lhsT=kT[pr][0:64, jsl],
                        rhs=qT[pr][0:64, islw],
                        start=True,
                        stop=True,
                        tile_position=(0, 0),
                    )
                    nc.tensor.matmul(
                        sb[:, 512 : 512 + w],
                        lhsT=kT[pr][64:128, jsl],
                        rhs=qT[pr][64:128, islw],
                        start=True,
                        stop=True,
                        tile_position=(64, 0),
                    )
                    nc.scalar.activation(
                        pTb[:, ioff : 512 + w],
                        sb[:, ioff : 512 + w],
                        EXP,
                        scale=0.125,
                    )
                    if dpos >= 0:
                        msl = mask_sb[:, 384 : 384 + w]
                        nc.vector.tensor_mul(
                            pTb[:, ioff:512], pTb[:, ioff:512], msl
                        )
                        nc.vector.tensor_mul(
                            pTb[:, 512 : 512 + w], pTb[:, 512 : 512 + w], msl
                        )
                    return ioff, w

                def pv_acc(pr, jt, pTb, ioff, w, ps_out, pacc0, pacc1):
                    first = jt == 0
                    last = jt == njt - 1
                    vt = v_sb[jt]
                    pT0 = pTb[:, ioff:512]
                    pT1 = pTb[:, 512 : 512 + w]
                    nc.tensor.matmul(
                        ps_out[0:64, ioff:512],
                        lhsT=vt[:, pr * 128 : pr * 128 + 64],
                        rhs=pT0,
                        start=first,
                        stop=False,
                        tile_position=(0, 0),
                        skip_group_check=True,
                    )
                    nc.tensor.matmul(
                        ps_out[64:128, ioff:512],
                        lhsT=vt[:, pr * 128 + 64 : pr * 128 + 128],
                        rhs=pT1,
                        start=first,
                        stop=last,
                        tile_position=(0, 64),
                        skip_group_check=True,
                    )
                    if first:
                        nc.vector.tensor_copy(pacc0, pTb[:, 0:512])
                        nc.vector.tensor_copy(pacc1, pTb[:, 512:1024])
                    else:
                        nc.vector.tensor_add(
                            pacc0[:, ioff:512], pacc0[:, ioff:512], pT0
                        )
                        nc.vector.tensor_add(
                            pacc1[:, ioff:512], pacc1[:, ioff:512], pT1
                        )

                for g in range(2):
                    duo = (2 * g, 2 * g + 1)
                    ps_outs = {}
                    paccs = {}
                    for pr in duo:
                        ps_outs[pr] = ps_o.tile(
                            [128, 512], f32, name="ps_out", tag="pso"
                        )
                        paccs[pr] = (
                            qpool.tile([128, 512], f16, name="pacc0", tag="pacc"),
                            qpool.tile([128, 512], f16, name="pacc1", tag="pacc"),
                        )
                    for jt in range(njt):
                        for pr in duo:
                            sb = ps_s.tile([128, 1024], f32, name="sb", tag="pss")
                            pTb = ppool.tile(
                                [128, 1024], f16, name="pTb", tag="pT"
                            )
                            ioff, w = qk_exp(pr, jt, sb, pTb)
                            pv_acc(pr, jt, pTb, ioff, w, ps_outs[pr], *paccs[pr])
                    for pr in duo:
                        # Partition-reduce the accumulated p-sums into rows
                        # 0/32 of a retired score-ring slot (frees a
                        # dedicated denominator bank).
                        den = ps_s.tile([128, 1024], f32, name="den", tag="pss")
                        nc.tensor.matmul(
                            den[0:1, 0:512],
                            lhsT=ones_sb,
                            rhs=paccs[pr][0],
                            start=True,
                            stop=True,
                            tile_position=(0, 0),
                            skip_group_check=True,
                        )
                        nc.tensor.matmul(
                            den[32:33, 0:512],
                            lhsT=ones_sb,
                            rhs=paccs[pr][1],
                            start=True,
                            stop=True,
                            tile_position=(0, 32),
                            skip_group_check=True,
                        )
                        # Unnormalized copy frees ps_out quickly;
                        # normalization happens in-place on att once the
                        # broadcast lands.
                        asl = att[pr][:, isl]
                        nc.vector.tensor_copy(asl, ps_outs[pr])
                        norm_pair(pr, den)

            def phase_c(s):
                """Output projection for token tiles 4s..4s+4."""
                for tt in range(4 * s, 4 * s + 4):
                    tsl = slice(tt * 128, (tt + 1) * 128)
                    for n in range(2):
                        ps = ps_a.tile([128, 512], f32, name="ps_c", tag="psA")
                        for fc in range(4):
                            nc.tensor.matmul(
                                ps,
                                lhsT=att[fc][:, tsl],
                                rhs=wo_sb[fc][:, n * 512 : (n + 1) * 512],
                                start=(fc == 0),
                                stop=(fc == 3),
                            )
                        ost = opool.tile([128, 512], f16, name="ost", tag="ost")
                        nc.vector.tensor_copy(ost, ps)
                        nc.sync.dma_start(
                            out=out[tsl, n * 512 : (n + 1) * 512], in_=ost
                        )

            # Emission order interleaves so every B phase has PE filler
            # work available: B(s) overlaps A(s+1) (emitted just before)
            # and C(s-1) (emitted just after B(s) starts). B(3), the
            # longest attention chunk, gets C(2)+C(3) as filler.
            phase_a(0)
            phase_b(0)
            phase_a(1)
            phase_b(1)
            phase_c(0)
            phase_a(2)
            phase_b(2)
            phase_c(1)
            phase_a(3)
            phase_b(3)
            phase_c(2)
            phase_c(3)

    nc.compile()
    return nc


def _get_program():
    if "nc" not in _CACHE:
        _CACHE["nc"] = _build_program()
    return _CACHE["nc"]


def _make_mask():
    # msk[jj, z] = 1 if z >= jj + 384 else 0; diagonal-position-p mask
    # tile is msk[:, 384-128p : 384-128p+512].
    jj = np.arange(128)[:, None]
    z = np.arange(896)[None, :]
    return (z >= jj + 384).astype(np.float16)


def _make_in_maps(x, w_qkv, w_out):
    mask = _make_mask()
    in_maps = []
    for core in range(NCORES):
        b, hg = core // 2, core % 2
        cs = slice(hg * 512, (hg + 1) * 512)
        f16 = np.float16
        in_maps.append(
            {
                "xt": np.ascontiguousarray(x[b].T).astype(f16),
                "wq": np.ascontiguousarray(
                    w_qkv[:, hg * 512 : hg * 512 + 512]
                ).astype(f16),
                "wk": np.ascontiguousarray(
                    w_qkv[:, 1024 + hg * 512 : 1024 + hg * 512 + 512]
                ).astype(f16),
                "wv": np.ascontiguousarray(
                    w_qkv[:, 2048 + hg * 512 : 2048 + hg * 512 + 512]
                ).astype(f16),
                "wo": np.ascontiguousarray(w_out[cs, :]).astype(f16),
                "msk": mask,
            }
        )
    return in_maps


def _run_device(in_maps, trace=False):
    from concourse.bass_utils import run_bass_kernel_spmd

    nc = _get_program()
    return run_bass_kernel_spmd(
        nc, in_maps, core_ids=list(range(NCORES)), trace=trace
    )


def kernel(x, w_qkv, w_out, b_out):
    x = np.asarray(x, dtype=np.float32)
    w_qkv = np.asarray(w_qkv, dtype=np.float32)
    w_out = np.asarray(w_out, dtype=np.float32)
    b_out = np.asarray(b_out, dtype=np.float32)

    res = _run_device(_make_in_maps(x, w_qkv, w_out)).results
    out = np.empty((B, T, C), dtype=np.float32)
    for b in range(B):
        out[b] = res[2 * b]["out"] + res[2 * b + 1]["out"] + b_out
    return out

